# revision 47
# baseline (speedup 1.0000x reference)
"""Trainium2 Bass kernel for CausalWanSelfAttention (KV-cache-bias attention).

Math: the reference's disjoint-segment attention + LSE merge is exactly
global softmax with a per-key bias b_l (log 0.1 on keys in
[frame_seqlen, current_block_start)).  exp needs no max-subtraction
(scores ~ N(0,1), max ~ 6), so out = (E @ V) / (1^T E) with
E = exp(scale*S + b_l) — the bias folds into the exp as a per-partition
bias (partition = key index within the 128-chunk).

Sharding: 24 units = (head h in 0..11, q-half in {0,1}), 3 units per core.
Each unit: 1024 queries x 1 head x all 8192 keys, 64 key chunks of 128.

Device layout per unit (matmuls bf16, accumulate fp32 PSUM; all matmuls
stream 512 q-columns so PE runs long back-to-back bursts):
  A:    S^T[l 128, q 1024] = kt-chunk^T @ qt          (1 ldw + 2 MM N=512)
  exp:  E = exp(S^T * scale + bias_l) bf16            (1 ACT instr)
  B:    O^T[d 128, q 1024] += v-chunk^T @ E           (1 ldw + 2 MM)
  norm: DVE pre-reduces E pairs->quads->octs->unit acc (bf16 tree), then
        n[1, q 1024] = ones^T @ acc (+ last oct)      (4 MM per unit,
        written into an s-ring PSUM slot: no dedicated norm bank)
Final divide by n and the [d,q]->[q,d] transpose happen host-side on the
fp32 partials (exact).

Pipeline: B lags A by 2 iterations and the PSUM s-ring holds 3 buffers,
so the in-order PE queue always has independent work while ACT runs exp;
outputs evacuate per-half to shorten unit-boundary WAR stalls and the
end-of-kernel tail; the first unit loads in small leading pieces so
compute starts ~3us in.

Optional (off by default): N_DVE>0 moves exp for N_DVE chunks per unit
to the DVE via an exp-as-int-bits tensor_scalar (placed inside the
attention-bias segment where the softmax mass is ~100x smaller, so the
~2% RMS trick error is negligible).  Measured on HW the ACT engine has
slack, so N_DVE=0 is both faster and more accurate.
"""

import math
import os
import sys

for _p in ("/opt/trn_rl_repo",):
    if _p not in sys.path:
        sys.path.insert(0, _p)

import numpy as np
import ml_dtypes

import concourse.bass as bass
import concourse.mybir as mybir
import concourse.tile as tile
from concourse import bacc
from concourse.bass_utils import run_bass_kernel_spmd

BF16 = mybir.dt.bfloat16
F32 = mybir.dt.float32
I16 = mybir.dt.int16
NP_BF16 = ml_dtypes.bfloat16

B, LQ, LK, H, D = 1, 2048, 8192, 12, 128
N_CORES = 8
UNITS_PER_CORE = 3          # 24 units = 12 heads x 2 q-halves
QSPAN = 1024                # queries per unit
NLC = LK // 128             # 64 key chunks of 128
SCALE = 1.0 / math.sqrt(D)

# exp-as-int-bits constants (bf16 bit pattern of e^x ~= A*x + B)
EXP_A = 128.0 / math.log(2.0)          # 184.664
EXP_B = 16256.0 - 7.35                 # 127*128 minus centering correction

_CACHED = {}
ABLATE = "base"   # timing experiments only; "base" is the real kernel
TIME_LOOP = 1     # timing experiments only: hardware-loop the body N times
N_DVE = 0         # number of chunks per unit whose exp runs on the DVE
                  # (0: the ACT engine has real-HW slack to do all of them)
MM_N = 512        # matmul free-dim per instruction (512 or 1024)


def _pick_dve_chunks(fs, bs):
    """Choose which key-chunks compute exp on the DVE.  Prefer chunks fully
    inside the biased segment [fs, bs): their keys carry ~100x less softmax
    mass (weight 0.1), so the int-bits exp approximation error there is
    negligible in the merged output.  Spacing >= 3 keeps the pipeline's
    deferred-B/PSUM-ring assumptions valid; chunks near unit edges are
    excluded (B-defer and PSUM-start ordering)."""
    if N_DVE == 0:
        return frozenset()
    lo = max(4, -(-fs // 128))
    hi = min(bs // 128, NLC - 3)
    biased = list(range(lo, hi))
    picks = []
    if len(biased) >= 3:
        m = max(3, len(biased) // N_DVE)
        picks = biased[m // 2::m][:N_DVE]
    if len(picks) < N_DVE:
        for c in range(4, NLC - 3, 8):
            if len(picks) >= N_DVE:
                break
            if all(abs(c - p) >= 3 for p in picks):
                picks.append(c)
    return frozenset(picks[:N_DVE])


def _build_program(dve_set=frozenset()):
    nc = bacc.Bacc("TRN2", target_bir_lowering=False, debug=False,
                   enable_asserts=False)

    qt_d = nc.dram_tensor("qt", [UNITS_PER_CORE, 128, QSPAN], BF16,
                          kind="ExternalInput")
    kt_d = nc.dram_tensor("kt", [UNITS_PER_CORE, 128, LK], BF16,
                          kind="ExternalInput")
    vl_d = nc.dram_tensor("vl", [UNITS_PER_CORE, LK, 128], BF16,
                          kind="ExternalInput")
    bias_d = nc.dram_tensor("bias", [128, NLC], F32, kind="ExternalInput")
    biasb_d = nc.dram_tensor("biasb", [128, NLC], F32, kind="ExternalInput")
    ot_d = nc.dram_tensor("ot", [UNITS_PER_CORE, 128, QSPAN], F32,
                          kind="ExternalOutput")
    nm_d = nc.dram_tensor("nm", [UNITS_PER_CORE, 1, QSPAN], F32,
                          kind="ExternalOutput")

    qt_ap = qt_d.ap()
    kt_ap = kt_d.ap()
    # [u, (c p), d] -> [u, p, c, d]: partition = key index within chunk
    vl_ap = vl_d.ap().rearrange("u (c p) d -> u p c d", p=128)
    bias_ap = bias_d.ap()
    biasb_ap = biasb_d.ap()
    ot_ap = ot_d.ap()
    nm_ap = nm_d.ap()

    with tile.TileContext(nc) as tc:
        with (
            tc.tile_pool(name="kt_pool", bufs=2) as kt_pool,
            tc.tile_pool(name="vl_pool", bufs=2) as vl_pool,
            tc.tile_pool(name="qt_pool", bufs=2) as qt_pool,
            tc.tile_pool(name="cn_pool", bufs=1) as cn_pool,
            tc.tile_pool(name="e_pool", bufs=4) as e_pool,
            tc.tile_pool(name="ob_pool", bufs=2) as ob_pool,
            tc.tile_pool(name="s_pool", bufs=3, space="PSUM") as s_pool,
            tc.tile_pool(name="o_pool", bufs=1, space="PSUM") as o_pool,
        ):
            bias_t = cn_pool.tile([128, NLC], F32, name="bias_t")
            biasb_t = cn_pool.tile([128, NLC], F32, name="biasb_t")
            ones_t = cn_pool.tile([128, 1], BF16, name="ones_t")
            nc.vector.memset(ones_t[:], 1.0)

            def load_bias():
                nc.sync.dma_start(out=bias_t[:], in_=bias_ap)
                nc.sync.dma_start(out=biasb_t[:], in_=biasb_ap)

            import contextlib
            loop_cm = (tc.For_i(0, TIME_LOOP, 1) if TIME_LOOP > 1
                       else contextlib.nullcontext())

            # ablation switches (timing experiments only)
            do_exp = ABLATE not in ("noexp", "empty")
            do_b = ABLATE not in ("nob", "empty")
            do_any = ABLATE != "empty"
            dummy_e = None
            if not do_exp and do_any:
                dummy_e = cn_pool.tile([128, QSPAN], BF16, name="dummy_e")
                nc.vector.memset(dummy_e[:], 0.001)

            loaded = {}

            def load_unit(u, warm=False):
                # qt first (every chunk needs it), then k/v leading pieces
                # (compute starts as soon as they land), then the rest —
                # few DMAs per unit keeps the serialized HWDGE issue cost low
                qt = qt_pool.tile([128, QSPAN], BF16, name=f"qt_u{u}", tag="qt")
                nc.sync.dma_start(out=qt[:], in_=qt_ap[u])
                kt = kt_pool.tile([128, LK], BF16, name=f"kt_u{u}", tag="kt")
                vl = vl_pool.tile([128, NLC, 128], BF16,
                                  name=f"vl_u{u}", tag="vl")
                if warm:
                    # cold start: tiny leading pieces so chunk 0 can begin
                    # ~3us in, bias vectors next (needed by the first exp),
                    # then a vl piece covering the mid chunks, then the rest
                    k8, c8 = LK // 8, NLC // 8
                    nc.sync.dma_start(out=kt[:, :k8], in_=kt_ap[u][:, :k8])
                    nc.sync.dma_start(out=vl[:, :c8, :],
                                      in_=vl_ap[u][:, :c8, :])
                    load_bias()
                    nc.sync.dma_start(out=vl[:, c8:2 * c8, :],
                                      in_=vl_ap[u][:, c8:2 * c8, :])
                    nc.sync.dma_start(out=kt[:, k8:], in_=kt_ap[u][:, k8:])
                    nc.sync.dma_start(out=vl[:, 2 * c8:, :],
                                      in_=vl_ap[u][:, 2 * c8:, :])
                else:
                    k4, c4 = LK // 4, NLC // 4
                    nc.sync.dma_start(out=kt[:, :k4], in_=kt_ap[u][:, :k4])
                    nc.sync.dma_start(out=vl[:, :c4, :],
                                      in_=vl_ap[u][:, :c4, :])
                    nc.sync.dma_start(out=kt[:, k4:], in_=kt_ap[u][:, k4:])
                    nc.sync.dma_start(out=vl[:, c4:, :],
                                      in_=vl_ap[u][:, c4:, :])
                loaded[u] = (kt, vl, qt)

            NG = UNITS_PER_CORE * NLC

            with loop_cm:
                if do_any:
                    load_unit(0, warm=True)
                else:
                    load_bias()
                # One global software-pipelined chunk stream across all
                # units: emit A(g) before B(g-1) so PE's in-order queue
                # always has independent work while the exp(g-1) runs, and
                # the next unit's A-phase fills the previous unit's drain.
                cur, ot_t, nm_t = {}, {}, {}
                etiles, ptiles, qtiles, otiles = {}, {}, {}, {}
                dve_b = {}     # iteration -> (chunk, e tile): deferred B phases
                dve_exp_pend = {}   # iteration -> (chunk, s tile, cg)
                pend_oct = []  # oct reductions deferred past the DVE stretch
                otiles2 = {}   # per-unit final oct (kept out of the acc chain)
                for g in range(NG + 6 if do_any else 0):
                    if g < NG:
                        ug, cg = g // NLC, g % NLC
                        if cg == 0:
                            cur[ug] = loaded.pop(ug)
                        kt, vl, qt = cur[ug]
                        s = s_pool.tile([128, QSPAN], F32)
                        for half in range(QSPAN // MM_N):
                            sl = bass.ts(half, MM_N)
                            nc.tensor.matmul(
                                s[:, sl], lhsT=kt[:, bass.ts(cg, 128)],
                                rhs=qt[:, sl], start=True, stop=True)
                        is_dve = do_exp and cg in dve_set
                        if is_dve:
                            # exp on DVE (bf16 bits of e^x via mult+add) —
                            # emitted next iteration, after that block's DVE
                            # adds, so the strict-FIFO DVE never idles
                            # waiting for this A matmul to finish.  Its B
                            # matmuls are deferred 3 iterations so the
                            # in-order PE queue never waits on the DVE.
                            dve_exp_pend[g + 1] = (g, s, cg)
                        elif do_exp:
                            e = e_pool.tile([128, QSPAN], BF16)
                            nc.scalar.activation(
                                e[:], s[:],
                                mybir.ActivationFunctionType.Exp,
                                bias=bias_t[:, cg:cg + 1], scale=SCALE)
                            etiles[g] = e
                        else:
                            etiles[g] = dummy_e
                        if cg == 8 and ug + 1 < UNITS_PER_CORE:
                            load_unit(ug + 1)  # prefetch next unit's inputs

                    def emit_b(d, e):
                        ud, dl = d // NLC, d % NLC
                        for half in range(QSPAN // MM_N):
                            sl = bass.ts(half, MM_N)
                            nc.tensor.matmul(
                                ot_t[ud][:, sl], lhsT=cur[ud][1][:, dl, :],
                                rhs=e[:, sl],
                                start=(dl == 0), stop=(dl == NLC - 1))

                    d = g - 2               # chunk whose B phase is due
                    if 0 <= d < NG and do_b:
                        ud, dl = d // NLC, d % NLC
                        if dl == 0:
                            ot_t[ud] = o_pool.tile([128, QSPAN], F32,
                                                   name=f"ot_u{ud}", tag="ot")
                        if not (do_exp and dl in dve_set):
                            emit_b(d, etiles[d])
                        for item in dve_b.pop(g, []):
                            emit_b(*item)
                        if do_exp and dl % 2 == 1:
                            pp = e_pool.tile([128, QSPAN], BF16,
                                             tag="pp", name=f"pp_{d}")
                            nc.vector.tensor_add(
                                pp[:], etiles.pop(d - 1)[:], etiles[d][:])
                            ptiles[d // 2] = pp
                        if do_exp and dl % 4 == 3:
                            qq = e_pool.tile([128, QSPAN], BF16,
                                             tag="qq", bufs=12,
                                             name=f"qq_{d}")
                            nc.vector.tensor_add(
                                qq[:], ptiles.pop(d // 2 - 1)[:],
                                ptiles.pop(d // 2)[:])
                            qtiles[d // 4] = qq

                        if g in dve_exp_pend:
                            gd, sd, cgd = dve_exp_pend.pop(g)
                            e = e_pool.tile([128, QSPAN], BF16,
                                            tag="ed", bufs=12,
                                            name=f"ed_{gd}")
                            nc.vector.tensor_scalar(
                                e[:].bitcast(I16), sd[:],
                                EXP_A * SCALE,
                                biasb_t[:, cgd:cgd + 1],
                                mybir.AluOpType.mult,
                                mybir.AluOpType.add)
                            etiles[gd] = e
                            # B matmuls spread one-per-iteration past the
                            # DVE stretch, keeping in-stretch PE load low
                            kk = sorted(dve_set).index(cgd)
                            dl_tgt = min(max(dve_set) + 1 + 2 * kk, NLC - 2)
                            tgt = max((gd // NLC) * NLC + dl_tgt + 2, gd + 3)
                            dve_b.setdefault(tgt, []).append((gd, e))

                        def emit_oct(dd):
                            udd = dd // NLC
                            oo = e_pool.tile([128, QSPAN], BF16,
                                             tag="oo", name=f"oo_{dd}")
                            nc.vector.tensor_add(
                                oo[:], qtiles.pop(dd // 4 - 1)[:],
                                qtiles.pop(dd // 4)[:])
                            if dd % NLC == 7:
                                otiles[udd] = oo
                            elif dd % NLC == NLC - 1:
                                # last oct stays separate: norm issues one
                                # accumulating matmul pair per operand, so
                                # the first pair runs before the unit ends
                                otiles2[udd] = oo
                            else:
                                # running unit accumulator: acc += oct
                                na = e_pool.tile([128, QSPAN], BF16,
                                                 tag="acc", name=f"acc_{dd}")
                                nc.vector.tensor_add(
                                    na[:], otiles[udd][:], oo[:])
                                otiles[udd] = na

                        if do_exp and dl % 8 == 7:
                            # inside the DVE-exp stretch the DVE is near
                            # saturation: defer oct+acc reduction until after
                            # the stretch (qq tiles ride a deeper ring)
                            if dve_set and (min(dve_set) - 1 <= dl
                                            <= max(dve_set) + 2):
                                pend_oct.append(d)
                            else:
                                emit_oct(d)
                        if (do_exp and pend_oct
                                and (dl > max(dve_set) + 2
                                     or dl >= NLC - 2)):
                            emit_oct(pend_oct.pop(0))
                        if dl == NLC - 1:
                            while do_exp and pend_oct:
                                emit_oct(pend_oct.pop(0))
                            etiles.pop(d)
                            ot = ot_t.pop(ud)
                            ot_sb = ob_pool.tile([128, QSPAN], F32,
                                                 name=f"otsb_u{ud}",
                                                 tag="otsb")
                            # evacuate + store per half: shortens both the
                            # end-of-kernel tail and the WAR stall on the
                            # next unit's first B matmuls
                            for half in range(QSPAN // MM_N):
                                sl = bass.ts(half, MM_N)
                                nc.vector.tensor_scalar_add(
                                    ot_sb[:, sl], ot[:, sl], 0.0)
                                nc.sync.dma_start(out=ot_ap[ud][:, sl],
                                                  in_=ot_sb[:, sl])
                    n1 = g - 3              # unit whose norm part 1 is due
                    if (n1 >= 0 and n1 % NLC == NLC - 2 and n1 < NG
                            and do_b):
                        un = n1 // NLC
                        acc = otiles.pop(un) if do_exp else dummy_e
                        # norm accumulates in an s-ring slot (no dedicated
                        # PSUM bank); part 1 = everything but the last oct,
                        # emitted early so only part 2 sits in the tail
                        nm = s_pool.tile([128, QSPAN], F32, name=f"nm_u{un}",
                                         tag="s")
                        nm_t[un] = nm
                        for half in range(QSPAN // MM_N):
                            sl = bass.ts(half, MM_N)
                            nc.tensor.matmul(
                                nm[0:1, sl], lhsT=ones_t[:],
                                rhs=acc[:, sl], start=True, stop=False)
                    n = g - 4               # unit whose norm part 2 is due
                    if n >= 0 and n % NLC == NLC - 1 and n < NG and do_b:
                        un = n // NLC
                        last = otiles2.pop(un) if do_exp else dummy_e
                        nm = nm_t.pop(un)
                        for half in range(QSPAN // MM_N):
                            sl = bass.ts(half, MM_N)
                            nc.tensor.matmul(
                                nm[0:1, sl], lhsT=ones_t[:],
                                rhs=last[:, sl], start=False, stop=True)
                        nm_sb = ob_pool.tile([1, QSPAN], F32,
                                             name=f"nmsb_u{un}", tag="nmsb")
                        if un == UNITS_PER_CORE - 1:
                            # tail: ACT is idle by now, DVE still drains the
                            # ot evacuation — copy via ACT off the DVE queue
                            nc.scalar.copy(nm_sb[:], nm[0:1, :])
                        else:
                            nc.vector.tensor_scalar_add(
                                nm_sb[:], nm[0:1, :], 0.0)
                        nc.sync.dma_start(out=nm_ap[un], in_=nm_sb[:])

    nc.compile()
    return nc


def _get_program(fs=1536, bs=6144):
    key = (_pick_dve_chunks(fs, bs), ABLATE, TIME_LOOP)
    if key not in _CACHED:
        _CACHED[key] = _build_program(key[0])
    return _CACHED[key]


def _host_prep(q, k, v, frame_seqlen, current_block_start):
    fs = max(0, min(int(frame_seqlen), LK))
    bs = max(0, min(int(current_block_start), LK))
    logw = np.zeros(LK, np.float32)
    logw[fs:bs] = math.log(0.1)
    bias = np.ascontiguousarray(logw.reshape(NLC, 128).T)  # [128, NLC]
    biasb = (EXP_B + EXP_A * bias).astype(np.float32)

    q = np.asarray(q, dtype=np.float32)
    k = np.asarray(k, dtype=np.float32)
    v = np.asarray(v, dtype=np.float32)

    qT = np.ascontiguousarray(q[0].transpose(1, 2, 0)).astype(NP_BF16)  # [H,128,LQ]
    kT = np.ascontiguousarray(k[0].transpose(1, 2, 0)).astype(NP_BF16)  # [H,128,LK]
    vL = np.ascontiguousarray(v[0].transpose(1, 0, 2)).astype(NP_BF16)  # [H,LK,128]

    in_maps = []
    for i in range(N_CORES):
        units = [3 * i + uu for uu in range(UNITS_PER_CORE)]
        heads = [g // 2 for g in units]
        qhs = [g % 2 for g in units]
        in_maps.append({
            "qt": np.ascontiguousarray(
                np.stack([qT[h, :, qh * QSPAN:(qh + 1) * QSPAN]
                          for h, qh in zip(heads, qhs)])),
            "kt": np.ascontiguousarray(np.stack([kT[h] for h in heads])),
            "vl": np.ascontiguousarray(np.stack([vL[h] for h in heads])),
            "bias": bias,
            "biasb": biasb,
        })
    return in_maps


def _assemble(results):
    out = np.empty((B, LQ, H, D), np.float32)
    for i in range(N_CORES):
        ot = results[i]["ot"]   # [3, 128, 1024] unnormalized O^T
        nm = results[i]["nm"][:, 0]   # [3, 1024]
        for uu in range(UNITS_PER_CORE):
            g = 3 * i + uu
            h, qh = g // 2, g % 2
            out[0, qh * QSPAN:(qh + 1) * QSPAN, h, :] = (
                ot[uu] / nm[uu][None, :]).T
    return out


def kernel(q, k, v, frame_seqlen, current_block_start):
    fs = max(0, min(int(frame_seqlen), LK))
    bs = max(0, min(int(current_block_start), LK))
    nc = _get_program(fs, bs)
    in_maps = _host_prep(q, k, v, frame_seqlen, current_block_start)
    res = run_bass_kernel_spmd(nc, in_maps, core_ids=list(range(N_CORES)))
    return _assemble(res.results)


# revision 62
# speedup vs baseline: 1.0536x; 1.0536x over previous
"""Trainium2 Bass kernel for CausalWanSelfAttention (KV-cache-bias attention).

Math: the reference's disjoint-segment attention + LSE merge is exactly
global softmax with a per-key bias b_l (log 0.1 on keys in
[frame_seqlen, current_block_start)).  exp needs no max-subtraction
(scores ~ N(0,1), max ~ 6), so out = (E @ V) / (1^T E) with
E = exp(scale*S + b_l) — the bias folds into the exp as a per-partition
bias (partition = key index within the 128-chunk).

Sharding: 24 units = (head h in 0..11, q-half in {0,1}), 3 units per core.
Each unit: 1024 queries x 1 head x all 8192 keys, 64 key chunks of 128.

Device layout per unit (matmuls bf16, accumulate fp32 PSUM; all matmuls
stream 512 q-columns so PE runs long back-to-back bursts):
  A:    S^T[l 128, q 1024] = kt-chunk^T @ qt          (1 ldw + 2 MM N=512)
  exp:  E = exp(S^T * scale + bias_l) bf16            (1 ACT instr)
  B:    O^T[d 128, q 1024] += v-chunk^T @ E           (1 ldw + 2 MM)
  norm: DVE pre-reduces E pairs->quads->octs->unit acc (bf16 tree), then
        n[1, q 1024] = ones^T @ acc (+ last oct)      (4 MM per unit,
        written into an s-ring PSUM slot: no dedicated norm bank)
Final divide by n and the [d,q]->[q,d] transpose happen host-side on the
fp32 partials (exact).

Pipeline: B lags A by 2 iterations and the PSUM s-ring holds 3 buffers,
so the in-order PE queue always has independent work while ACT runs exp;
outputs evacuate per-half to shorten unit-boundary WAR stalls and the
end-of-kernel tail; the first unit loads in small leading pieces so
compute starts ~3us in.

Optional (off by default): N_DVE>0 moves exp for N_DVE chunks per unit
to the DVE via an exp-as-int-bits tensor_scalar (placed inside the
attention-bias segment where the softmax mass is ~100x smaller, so the
~2% RMS trick error is negligible).  Measured on HW the ACT engine has
slack, so N_DVE=0 is both faster and more accurate.
"""

import math
import os
import sys

for _p in ("/opt/trn_rl_repo",):
    if _p not in sys.path:
        sys.path.insert(0, _p)

import numpy as np
import ml_dtypes

import concourse.bass as bass
import concourse.mybir as mybir
import concourse.tile as tile
from concourse import bacc
from concourse.bass_utils import run_bass_kernel_spmd

BF16 = mybir.dt.bfloat16
F32 = mybir.dt.float32
I16 = mybir.dt.int16
FP8 = mybir.dt.float8e4
NP_BF16 = ml_dtypes.bfloat16
NP_FP8 = ml_dtypes.float8_e4m3

B, LQ, LK, H, D = 1, 2048, 8192, 12, 128
N_CORES = 8
UNITS_PER_CORE = 3          # 24 units = 12 heads x 2 q-halves
QSPAN = 1024                # queries per unit
NLC = LK // 128             # 64 key chunks of 128
SCALE = 1.0 / math.sqrt(D)

# exp-as-int-bits constants (bf16 bit pattern of e^x ~= A*x + B)
EXP_A = 128.0 / math.log(2.0)          # 184.664
EXP_B = 16256.0 - 7.35                 # 127*128 minus centering correction

_CACHED = {}
ABLATE = "base"   # timing experiments only; "base" is the real kernel
TIME_LOOP = 1     # timing experiments only: hardware-loop the body N times
N_DVE = 0         # number of chunks per unit whose exp runs on the DVE
                  # (0: the ACT engine has real-HW slack to do all of them)
MM_N = 512        # matmul free-dim per instruction (1024 fails NEFF load)
USE_FP8 = False   # fp8 DoubleRow B-phase on biased-segment chunk pairs:
                  # HW-validated correct (rel err 6.2e-3) but measured
                  # speed-neutral vs bf16 (DoubleRow saves matmuls yet pays
                  # wider ldweights + PE dtype-mode switches + 1x-mode fp8
                  # norm-tree adds), so bf16 wins on accuracy margin


def _pick_fp8_pairs(fs, bs):
    """Chunk pairs (c, c+1), c even, fully inside the biased segment: their
    keys carry ~100x less softmax mass (weight 0.1), so fp8e4m3 E/V rounding
    there is negligible in the merged output, and the DoubleRow fp8 matmul
    runs the B phase at 2 key-rows per cycle.  Pairs stay clear of unit
    edges so they never carry the PSUM start/stop accumulate flags."""
    if not USE_FP8:
        return ()
    lo = max(2, -(-fs // 128))
    hi = min(bs // 128, NLC - 2)
    lo += lo % 2
    return tuple(c for c in range(lo, hi - 1, 2))


def _pick_dve_chunks(fs, bs):
    """Choose which key-chunks compute exp on the DVE.  Prefer chunks fully
    inside the biased segment [fs, bs): their keys carry ~100x less softmax
    mass (weight 0.1), so the int-bits exp approximation error there is
    negligible in the merged output.  Spacing >= 3 keeps the pipeline's
    deferred-B/PSUM-ring assumptions valid; chunks near unit edges are
    excluded (B-defer and PSUM-start ordering)."""
    if N_DVE == 0:
        return frozenset()
    lo = max(4, -(-fs // 128))
    hi = min(bs // 128, NLC - 3)
    biased = list(range(lo, hi))
    picks = []
    if len(biased) >= 3:
        m = max(3, len(biased) // N_DVE)
        picks = biased[m // 2::m][:N_DVE]
    if len(picks) < N_DVE:
        for c in range(4, NLC - 3, 8):
            if len(picks) >= N_DVE:
                break
            if all(abs(c - p) >= 3 for p in picks):
                picks.append(c)
    return frozenset(picks[:N_DVE])


def _build_program(dve_set=frozenset(), fp8_pairs=()):
    nc = bacc.Bacc("TRN2", target_bir_lowering=False, debug=False,
                   enable_asserts=False)
    npair = len(fp8_pairs)
    pair_idx = {c: i for i, c in enumerate(fp8_pairs)}
    fp8_even = set(fp8_pairs)
    fp8_odd = {c + 1 for c in fp8_pairs}

    qt_d = nc.dram_tensor("qt", [UNITS_PER_CORE, 128, QSPAN], BF16,
                          kind="ExternalInput")
    kt_d = nc.dram_tensor("kt", [UNITS_PER_CORE, 128, LK], BF16,
                          kind="ExternalInput")
    vl_d = nc.dram_tensor("vl", [UNITS_PER_CORE, LK, 128], BF16,
                          kind="ExternalInput")
    vf_d = (nc.dram_tensor("vf", [UNITS_PER_CORE, 128, npair, 2, 128], FP8,
                           kind="ExternalInput") if npair else None)
    bias_d = nc.dram_tensor("bias", [128, NLC], F32, kind="ExternalInput")
    biasb_d = nc.dram_tensor("biasb", [128, NLC], F32, kind="ExternalInput")
    ot_d = nc.dram_tensor("ot", [UNITS_PER_CORE, 128, QSPAN], F32,
                          kind="ExternalOutput")
    nm_d = nc.dram_tensor("nm", [UNITS_PER_CORE, 1, QSPAN], F32,
                          kind="ExternalOutput")

    qt_ap = qt_d.ap()
    kt_ap = kt_d.ap()
    vf_ap = vf_d.ap() if npair else None
    # [u, (c p), d] -> [u, p, c, d]: partition = key index within chunk
    vl_ap = vl_d.ap().rearrange("u (c p) d -> u p c d", p=128)
    bias_ap = bias_d.ap()
    biasb_ap = biasb_d.ap()
    ot_ap = ot_d.ap()
    nm_ap = nm_d.ap()

    with tile.TileContext(nc) as tc:
        with (
            tc.tile_pool(name="kt_pool", bufs=2) as kt_pool,
            tc.tile_pool(name="vl_pool", bufs=2) as vl_pool,
            tc.tile_pool(name="vf_pool", bufs=2) as vf_pool,
            tc.tile_pool(name="qt_pool", bufs=2) as qt_pool,
            tc.tile_pool(name="cn_pool", bufs=1) as cn_pool,
            tc.tile_pool(name="e_pool", bufs=4) as e_pool,
            tc.tile_pool(name="ob_pool", bufs=2) as ob_pool,
            tc.tile_pool(name="s_pool", bufs=3, space="PSUM") as s_pool,
            tc.tile_pool(name="o_pool", bufs=1, space="PSUM") as o_pool,
        ):
            bias_t = cn_pool.tile([128, NLC], F32, name="bias_t")
            biasb_t = cn_pool.tile([128, NLC], F32, name="biasb_t")
            ones_t = cn_pool.tile([128, 1], BF16, name="ones_t")
            nc.vector.memset(ones_t[:], 1.0)

            def load_bias():
                nc.sync.dma_start(out=bias_t[:], in_=bias_ap)
                nc.sync.dma_start(out=biasb_t[:], in_=biasb_ap)

            import contextlib
            loop_cm = (tc.For_i(0, TIME_LOOP, 1) if TIME_LOOP > 1
                       else contextlib.nullcontext())

            # ablation switches (timing experiments only)
            do_exp = ABLATE not in ("noexp", "empty")
            do_b = ABLATE not in ("nob", "empty")
            do_any = ABLATE != "empty"
            dummy_e = None
            if not do_exp and do_any:
                dummy_e = cn_pool.tile([128, QSPAN], BF16, name="dummy_e")
                nc.vector.memset(dummy_e[:], 0.001)

            loaded = {}

            def load_unit(u, warm=False):
                # qt first (every chunk needs it), then k/v leading pieces
                # (compute starts as soon as they land), then the rest —
                # few DMAs per unit keeps the serialized HWDGE issue cost low
                qt = qt_pool.tile([128, QSPAN], BF16, name=f"qt_u{u}", tag="qt")
                nc.sync.dma_start(out=qt[:], in_=qt_ap[u])
                kt = kt_pool.tile([128, LK], BF16, name=f"kt_u{u}", tag="kt")
                vl = vl_pool.tile([128, NLC, 128], BF16,
                                  name=f"vl_u{u}", tag="vl")
                vf = (vf_pool.tile([128, npair, 2, 128], FP8,
                                   name=f"vf_u{u}", tag="vf")
                      if npair else None)
                if warm:
                    # cold start: tiny leading pieces so chunk 0 can begin
                    # ~3us in, bias vectors next (needed by the first exp),
                    # then a vl piece covering the mid chunks, then the rest
                    k8, c8 = LK // 8, NLC // 8
                    nc.sync.dma_start(out=kt[:, :k8], in_=kt_ap[u][:, :k8])
                    nc.sync.dma_start(out=vl[:, :c8, :],
                                      in_=vl_ap[u][:, :c8, :])
                    load_bias()
                    if npair:
                        nc.sync.dma_start(out=vf[:], in_=vf_ap[u])
                    nc.sync.dma_start(out=vl[:, c8:2 * c8, :],
                                      in_=vl_ap[u][:, c8:2 * c8, :])
                    nc.sync.dma_start(out=kt[:, k8:], in_=kt_ap[u][:, k8:])
                    nc.sync.dma_start(out=vl[:, 2 * c8:, :],
                                      in_=vl_ap[u][:, 2 * c8:, :])
                else:
                    k4, c4 = LK // 4, NLC // 4
                    nc.sync.dma_start(out=kt[:, :k4], in_=kt_ap[u][:, :k4])
                    nc.sync.dma_start(out=vl[:, :c4, :],
                                      in_=vl_ap[u][:, :c4, :])
                    if npair:
                        nc.sync.dma_start(out=vf[:], in_=vf_ap[u])
                    nc.sync.dma_start(out=kt[:, k4:], in_=kt_ap[u][:, k4:])
                    nc.sync.dma_start(out=vl[:, c4:, :],
                                      in_=vl_ap[u][:, c4:, :])
                loaded[u] = (kt, vl, qt, vf)

            NG = UNITS_PER_CORE * NLC

            with loop_cm:
                if do_any:
                    load_unit(0, warm=True)
                else:
                    load_bias()
                # One global software-pipelined chunk stream across all
                # units: emit A(g) before B(g-1) so PE's in-order queue
                # always has independent work while the exp(g-1) runs, and
                # the next unit's A-phase fills the previous unit's drain.
                cur, ot_t, nm_t = {}, {}, {}
                etiles, ptiles, qtiles, otiles = {}, {}, {}, {}
                dve_b = {}     # iteration -> (chunk, e tile): deferred B phases
                dve_exp_pend = {}   # iteration -> (chunk, s tile, cg)
                pend_oct = []  # oct reductions deferred past the DVE stretch
                otiles2 = {}   # per-unit final oct (kept out of the acc chain)
                cur_ef = {}    # per-unit open fp8 pair tile
                fp8_b = {}     # iteration -> (pair tile, pair index)
                for g in range(NG + 6 if do_any else 0):
                    if g < NG:
                        ug, cg = g // NLC, g % NLC
                        if cg == 0:
                            cur[ug] = loaded.pop(ug)
                        kt, vl, qt, vf = cur[ug]
                        s = s_pool.tile([128, QSPAN], F32)
                        for half in range(QSPAN // MM_N):
                            sl = bass.ts(half, MM_N)
                            nc.tensor.matmul(
                                s[:, sl], lhsT=kt[:, bass.ts(cg, 128)],
                                rhs=qt[:, sl], start=True, stop=True)
                        is_dve = do_exp and cg in dve_set
                        if is_dve:
                            # exp on DVE (bf16 bits of e^x via mult+add) —
                            # emitted next iteration, after that block's DVE
                            # adds, so the strict-FIFO DVE never idles
                            # waiting for this A matmul to finish.  Its B
                            # matmuls are deferred 3 iterations so the
                            # in-order PE queue never waits on the DVE.
                            dve_exp_pend[g + 1] = (g, s, cg)
                        elif do_exp and cg in fp8_even:
                            # biased pair: exp straight to fp8e4 halves of a
                            # [128, 2, q] pair tile for the DoubleRow B MM
                            ef = e_pool.tile([128, 2, QSPAN], FP8,
                                             tag="ef", name=f"ef_{g}")
                            cur_ef[ug] = ef
                            nc.scalar.activation(
                                ef[:, 0, :], s[:],
                                mybir.ActivationFunctionType.Exp,
                                bias=bias_t[:, cg:cg + 1], scale=SCALE)
                            etiles[g] = ef[:, 0, :]
                        elif do_exp and cg in fp8_odd:
                            ef = cur_ef[ug]
                            nc.scalar.activation(
                                ef[:, 1, :], s[:],
                                mybir.ActivationFunctionType.Exp,
                                bias=bias_t[:, cg:cg + 1], scale=SCALE)
                            etiles[g] = ef[:, 1, :]
                            fp8_b[g + 2] = (ef, pair_idx[cg - 1])
                        elif do_exp:
                            e = e_pool.tile([128, QSPAN], BF16)
                            nc.scalar.activation(
                                e[:], s[:],
                                mybir.ActivationFunctionType.Exp,
                                bias=bias_t[:, cg:cg + 1], scale=SCALE)
                            etiles[g] = e
                        else:
                            etiles[g] = dummy_e
                        if cg == 8 and ug + 1 < UNITS_PER_CORE:
                            load_unit(ug + 1)  # prefetch next unit's inputs

                    def emit_b(d, e):
                        ud, dl = d // NLC, d % NLC
                        for half in range(QSPAN // MM_N):
                            sl = bass.ts(half, MM_N)
                            nc.tensor.matmul(
                                ot_t[ud][:, sl], lhsT=cur[ud][1][:, dl, :],
                                rhs=e[:, sl],
                                start=(dl == 0), stop=(dl == NLC - 1))

                    d = g - 2               # chunk whose B phase is due
                    if 0 <= d < NG and do_b:
                        ud, dl = d // NLC, d % NLC
                        if dl == 0:
                            ot_t[ud] = o_pool.tile([128, QSPAN], F32,
                                                   name=f"ot_u{ud}", tag="ot")
                        if not (do_exp and (dl in dve_set or dl in fp8_even
                                            or dl in fp8_odd)):
                            emit_b(d, etiles[d])
                        if g in fp8_b:
                            # one DoubleRow MM pair covers both fp8 chunks:
                            # contraction over (partition, j) = 256 keys at
                            # 2 key-rows/cycle
                            ef, pi = fp8_b.pop(g)
                            for half in range(QSPAN // MM_N):
                                sl = bass.ts(half, MM_N)
                                nc.tensor.matmul(
                                    ot_t[ud][:, sl],
                                    lhsT=cur[ud][3][:, pi, :, :],
                                    rhs=ef[:, :, sl],
                                    start=False, stop=False,
                                    perf_mode=mybir.MatmulPerfMode.DoubleRow)
                        for item in dve_b.pop(g, []):
                            emit_b(*item)
                        if do_exp and dl % 2 == 1:
                            pp = e_pool.tile([128, QSPAN], BF16,
                                             tag="pp", name=f"pp_{d}")
                            nc.vector.tensor_add(
                                pp[:], etiles.pop(d - 1)[:], etiles[d][:])
                            ptiles[d // 2] = pp
                        if do_exp and dl % 4 == 3:
                            qq = e_pool.tile([128, QSPAN], BF16,
                                             tag="qq", bufs=12,
                                             name=f"qq_{d}")
                            nc.vector.tensor_add(
                                qq[:], ptiles.pop(d // 2 - 1)[:],
                                ptiles.pop(d // 2)[:])
                            qtiles[d // 4] = qq

                        if g in dve_exp_pend:
                            gd, sd, cgd = dve_exp_pend.pop(g)
                            e = e_pool.tile([128, QSPAN], BF16,
                                            tag="ed", bufs=12,
                                            name=f"ed_{gd}")
                            nc.vector.tensor_scalar(
                                e[:].bitcast(I16), sd[:],
                                EXP_A * SCALE,
                                biasb_t[:, cgd:cgd + 1],
                                mybir.AluOpType.mult,
                                mybir.AluOpType.add)
                            etiles[gd] = e
                            # B matmuls spread one-per-iteration past the
                            # DVE stretch, keeping in-stretch PE load low
                            kk = sorted(dve_set).index(cgd)
                            dl_tgt = min(max(dve_set) + 1 + 2 * kk, NLC - 2)
                            tgt = max((gd // NLC) * NLC + dl_tgt + 2, gd + 3)
                            dve_b.setdefault(tgt, []).append((gd, e))

                        def emit_oct(dd):
                            udd = dd // NLC
                            oo = e_pool.tile([128, QSPAN], BF16,
                                             tag="oo", name=f"oo_{dd}")
                            nc.vector.tensor_add(
                                oo[:], qtiles.pop(dd // 4 - 1)[:],
                                qtiles.pop(dd // 4)[:])
                            if dd % NLC == 7:
                                otiles[udd] = oo
                            elif dd % NLC == NLC - 1:
                                # last oct stays separate: norm issues one
                                # accumulating matmul pair per operand, so
                                # the first pair runs before the unit ends
                                otiles2[udd] = oo
                            else:
                                # running unit accumulator: acc += oct
                                na = e_pool.tile([128, QSPAN], BF16,
                                                 tag="acc", name=f"acc_{dd}")
                                nc.vector.tensor_add(
                                    na[:], otiles[udd][:], oo[:])
                                otiles[udd] = na

                        if do_exp and dl % 8 == 7:
                            # inside the DVE-exp stretch the DVE is near
                            # saturation: defer oct+acc reduction until after
                            # the stretch (qq tiles ride a deeper ring)
                            if dve_set and (min(dve_set) - 1 <= dl
                                            <= max(dve_set) + 2):
                                pend_oct.append(d)
                            else:
                                emit_oct(d)
                        if (do_exp and pend_oct
                                and (dl > max(dve_set) + 2
                                     or dl >= NLC - 2)):
                            emit_oct(pend_oct.pop(0))
                        if dl == NLC - 1:
                            while do_exp and pend_oct:
                                emit_oct(pend_oct.pop(0))
                            etiles.pop(d)
                            ot = ot_t.pop(ud)
                            ot_sb = ob_pool.tile([128, QSPAN], F32,
                                                 name=f"otsb_u{ud}",
                                                 tag="otsb")
                            # evacuate + store per half: shortens both the
                            # end-of-kernel tail and the WAR stall on the
                            # next unit's first B matmuls
                            for half in range(QSPAN // MM_N):
                                sl = bass.ts(half, MM_N)
                                nc.vector.tensor_scalar_add(
                                    ot_sb[:, sl], ot[:, sl], 0.0)
                                nc.sync.dma_start(out=ot_ap[ud][:, sl],
                                                  in_=ot_sb[:, sl])
                    n1 = g - 3              # unit whose norm part 1 is due
                    if (n1 >= 0 and n1 % NLC == NLC - 2 and n1 < NG
                            and do_b):
                        un = n1 // NLC
                        acc = otiles.pop(un) if do_exp else dummy_e
                        # norm accumulates in an s-ring slot (no dedicated
                        # PSUM bank); part 1 = everything but the last oct,
                        # emitted early so only part 2 sits in the tail
                        nm = s_pool.tile([128, QSPAN], F32, name=f"nm_u{un}",
                                         tag="s")
                        nm_t[un] = nm
                        for half in range(QSPAN // MM_N):
                            sl = bass.ts(half, MM_N)
                            nc.tensor.matmul(
                                nm[0:1, sl], lhsT=ones_t[:],
                                rhs=acc[:, sl], start=True, stop=False)
                    n = g - 4               # unit whose norm part 2 is due
                    if n >= 0 and n % NLC == NLC - 1 and n < NG and do_b:
                        un = n // NLC
                        last = otiles2.pop(un) if do_exp else dummy_e
                        nm = nm_t.pop(un)
                        for half in range(QSPAN // MM_N):
                            sl = bass.ts(half, MM_N)
                            nc.tensor.matmul(
                                nm[0:1, sl], lhsT=ones_t[:],
                                rhs=last[:, sl], start=False, stop=True)
                        nm_sb = ob_pool.tile([1, QSPAN], F32,
                                             name=f"nmsb_u{un}", tag="nmsb")
                        if un == UNITS_PER_CORE - 1:
                            # tail: ACT is idle by now, DVE still drains the
                            # ot evacuation — copy via ACT off the DVE queue
                            nc.scalar.copy(nm_sb[:], nm[0:1, :])
                        else:
                            nc.vector.tensor_scalar_add(
                                nm_sb[:], nm[0:1, :], 0.0)
                        nc.sync.dma_start(out=nm_ap[un], in_=nm_sb[:])

    nc.compile()
    return nc


def _get_program(fs=1536, bs=6144):
    key = (_pick_dve_chunks(fs, bs), _pick_fp8_pairs(fs, bs),
           ABLATE, TIME_LOOP, MM_N)
    if key not in _CACHED:
        _CACHED[key] = _build_program(key[0], key[1])
    return _CACHED[key]


def _host_prep(q, k, v, frame_seqlen, current_block_start):
    fs = max(0, min(int(frame_seqlen), LK))
    bs = max(0, min(int(current_block_start), LK))
    logw = np.zeros(LK, np.float32)
    logw[fs:bs] = math.log(0.1)
    bias = np.ascontiguousarray(logw.reshape(NLC, 128).T)  # [128, NLC]
    biasb = (EXP_B + EXP_A * bias).astype(np.float32)

    q = np.asarray(q, dtype=np.float32)
    k = np.asarray(k, dtype=np.float32)
    v = np.asarray(v, dtype=np.float32)

    qT = np.ascontiguousarray(q[0].transpose(1, 2, 0)).astype(NP_BF16)  # [H,128,LQ]
    kT = np.ascontiguousarray(k[0].transpose(1, 2, 0)).astype(NP_BF16)  # [H,128,LK]
    vL = np.ascontiguousarray(v[0].transpose(1, 0, 2)).astype(NP_BF16)  # [H,LK,128]

    pairs = list(_pick_fp8_pairs(fs, bs))
    vF = None
    if pairs:
        # [H, 128(l), npair, 2, 128(d)] fp8 pair layout for DoubleRow lhsT
        vch = v[0].reshape(NLC, 128, H, D).transpose(2, 0, 1, 3)  # [H,c,l,d]
        vF = np.stack([vch[:, pairs, :, :], vch[:, [c + 1 for c in pairs]]],
                      axis=3)                   # [H, npair, l, 2, d]
        vF = np.ascontiguousarray(vF.transpose(0, 2, 1, 3, 4)).astype(NP_FP8)

    in_maps = []
    for i in range(N_CORES):
        units = [3 * i + uu for uu in range(UNITS_PER_CORE)]
        heads = [g // 2 for g in units]
        qhs = [g % 2 for g in units]
        m = {
            "qt": np.ascontiguousarray(
                np.stack([qT[h, :, qh * QSPAN:(qh + 1) * QSPAN]
                          for h, qh in zip(heads, qhs)])),
            "kt": np.ascontiguousarray(np.stack([kT[h] for h in heads])),
            "vl": np.ascontiguousarray(np.stack([vL[h] for h in heads])),
            "bias": bias,
            "biasb": biasb,
        }
        if pairs:
            m["vf"] = np.ascontiguousarray(np.stack([vF[h] for h in heads]))
        in_maps.append(m)
    return in_maps


def _assemble(results):
    out = np.empty((B, LQ, H, D), np.float32)
    for i in range(N_CORES):
        ot = results[i]["ot"]   # [3, 128, 1024] unnormalized O^T
        nm = results[i]["nm"][:, 0]   # [3, 1024]
        for uu in range(UNITS_PER_CORE):
            g = 3 * i + uu
            h, qh = g // 2, g % 2
            out[0, qh * QSPAN:(qh + 1) * QSPAN, h, :] = (
                ot[uu] / nm[uu][None, :]).T
    return out


def kernel(q, k, v, frame_seqlen, current_block_start):
    fs = max(0, min(int(frame_seqlen), LK))
    bs = max(0, min(int(current_block_start), LK))
    nc = _get_program(fs, bs)
    in_maps = _host_prep(q, k, v, frame_seqlen, current_block_start)
    res = run_bass_kernel_spmd(nc, in_maps, core_ids=list(range(N_CORES)))
    return _assemble(res.results)


# revision 76
# speedup vs baseline: 1.4875x; 1.4118x over previous
"""Trainium2 Bass kernel for CausalWanSelfAttention (KV-cache-bias attention).

Math: the reference's disjoint-segment attention + LSE merge is exactly
global softmax with a per-key bias b_l (log 0.1 on keys in
[frame_seqlen, current_block_start)).  exp needs no max-subtraction
(scores ~ N(0,1), max ~ 6), so out = (E @ V) / (1^T E) with
E = exp(scale*S + b_l) — the bias folds into the exp as a per-partition
bias (partition = key index within the 128-chunk).

Sharding: 24 units = (head h in 0..11, q-half in {0,1}), 3 units per core.
Each unit: 1024 queries x 1 head x all 8192 keys, 64 key chunks of 128.

Device layout per unit (matmuls bf16, accumulate fp32 PSUM; all matmuls
stream 512 q-columns so PE runs long back-to-back bursts):
  A:    S^T[l 128, q 1024] = kt-chunk^T @ qt          (1 ldw + 2 MM N=512)
  exp:  E = exp(S^T * scale + bias_l) bf16            (1 ACT instr)
  B:    O^T[d 128, q 1024] += v-chunk^T @ E           (1 ldw + 2 MM)
  norm: DVE pre-reduces E pairs->quads->octs->unit acc (bf16 tree), then
        n[1, q 1024] = ones^T @ acc (+ last oct)      (4 MM per unit,
        written into an s-ring PSUM slot: no dedicated norm bank)
Final divide by n and the [d,q]->[q,d] transpose happen host-side on the
fp32 partials (exact).

Pipeline: B lags A by 2 iterations and the PSUM s-ring holds 3 buffers,
so the in-order PE queue always has independent work while ACT runs exp;
outputs evacuate per-half to shorten unit-boundary WAR stalls and the
end-of-kernel tail; the first unit loads in small leading pieces so
compute starts ~3us in.

Optional (off by default): N_DVE>0 moves exp for N_DVE chunks per unit
to the DVE via an exp-as-int-bits tensor_scalar (placed inside the
attention-bias segment where the softmax mass is ~100x smaller, so the
~2% RMS trick error is negligible).  Measured on HW the ACT engine has
slack, so N_DVE=0 is both faster and more accurate.
"""

import math
import os
import sys

for _p in ("/opt/trn_rl_repo",):
    if _p not in sys.path:
        sys.path.insert(0, _p)

import numpy as np
import ml_dtypes

import concourse.bass as bass
import concourse.mybir as mybir
import concourse.tile as tile
from concourse import bacc
from concourse.bass_utils import run_bass_kernel_spmd

BF16 = mybir.dt.bfloat16
F32 = mybir.dt.float32
I16 = mybir.dt.int16
FP8 = mybir.dt.float8e4
NP_BF16 = ml_dtypes.bfloat16
NP_FP8 = ml_dtypes.float8_e4m3

B, LQ, LK, H, D = 1, 2048, 8192, 12, 128
N_CORES = 8
UNITS_PER_CORE = 3          # 24 units = 12 heads x 2 q-halves
QSPAN = 1024                # queries per unit
NLC = LK // 128             # 64 key chunks of 128
SCALE = 1.0 / math.sqrt(D)

# exp-as-int-bits constants (bf16 bit pattern of e^x ~= A*x + B)
EXP_A = 128.0 / math.log(2.0)          # 184.664
EXP_B = 16256.0 - 7.35                 # 127*128 minus centering correction

_CACHED = {}
ABLATE = "base"   # timing experiments only; "base" is the real kernel
TIME_LOOP = 1     # timing experiments only: hardware-loop the body N times
N_DVE = 0         # number of chunks per unit whose exp runs on the DVE
                  # (0: the ACT engine has real-HW slack to do all of them)
MM_N = 512        # matmul free-dim per instruction (1024 fails NEFF load)
USE_FP8 = False   # fp8 DoubleRow B-phase on biased-segment chunk pairs:
                  # HW-validated correct (rel err 6.2e-3) but measured
                  # speed-neutral vs bf16 (DoubleRow saves matmuls yet pays
                  # wider ldweights + PE dtype-mode switches + 1x-mode fp8
                  # norm-tree adds), so bf16 wins on accuracy margin


def _pick_fp8_pairs(fs, bs):
    """Chunk pairs (c, c+1), c even, fully inside the biased segment: their
    keys carry ~100x less softmax mass (weight 0.1), so fp8e4m3 E/V rounding
    there is negligible in the merged output, and the DoubleRow fp8 matmul
    runs the B phase at 2 key-rows per cycle.  Pairs stay clear of unit
    edges so they never carry the PSUM start/stop accumulate flags."""
    if not USE_FP8:
        return ()
    lo = max(2, -(-fs // 128))
    hi = min(bs // 128, NLC - 2)
    lo += lo % 2
    return tuple(c for c in range(lo, hi - 1, 2))


def _pick_dve_chunks(fs, bs):
    """Choose which key-chunks compute exp on the DVE.  Prefer chunks fully
    inside the biased segment [fs, bs): their keys carry ~100x less softmax
    mass (weight 0.1), so the int-bits exp approximation error there is
    negligible in the merged output.  Spacing >= 3 keeps the pipeline's
    deferred-B/PSUM-ring assumptions valid; chunks near unit edges are
    excluded (B-defer and PSUM-start ordering)."""
    if N_DVE == 0:
        return frozenset()
    lo = max(4, -(-fs // 128))
    hi = min(bs // 128, NLC - 3)
    biased = list(range(lo, hi))
    picks = []
    if len(biased) >= 3:
        m = max(3, len(biased) // N_DVE)
        picks = biased[m // 2::m][:N_DVE]
    if len(picks) < N_DVE:
        for c in range(4, NLC - 3, 8):
            if len(picks) >= N_DVE:
                break
            if all(abs(c - p) >= 3 for p in picks):
                picks.append(c)
    return frozenset(picks[:N_DVE])


def _build_program(dve_set=frozenset(), fp8_pairs=()):
    nc = bacc.Bacc("TRN2", target_bir_lowering=False, debug=False,
                   enable_asserts=False)
    npair = len(fp8_pairs)
    pair_idx = {c: i for i, c in enumerate(fp8_pairs)}
    fp8_even = set(fp8_pairs)
    fp8_odd = {c + 1 for c in fp8_pairs}

    qt_d = nc.dram_tensor("qt", [UNITS_PER_CORE, 128, QSPAN], BF16,
                          kind="ExternalInput")
    kt_d = nc.dram_tensor("kt", [UNITS_PER_CORE, 128, LK], BF16,
                          kind="ExternalInput")
    vl_d = nc.dram_tensor("vl", [UNITS_PER_CORE, LK, 128], BF16,
                          kind="ExternalInput")
    vf_d = (nc.dram_tensor("vf", [UNITS_PER_CORE, 128, npair, 2, 128], FP8,
                           kind="ExternalInput") if npair else None)
    bias_d = nc.dram_tensor("bias", [128, NLC], F32, kind="ExternalInput")
    biasb_d = nc.dram_tensor("biasb", [128, NLC], F32, kind="ExternalInput")
    ot_d = nc.dram_tensor("ot", [UNITS_PER_CORE, 128, QSPAN], F32,
                          kind="ExternalOutput")
    nm_d = nc.dram_tensor("nm", [UNITS_PER_CORE, 1, QSPAN], F32,
                          kind="ExternalOutput")

    qt_ap = qt_d.ap()
    kt_ap = kt_d.ap()
    vf_ap = vf_d.ap() if npair else None
    # [u, (c p), d] -> [u, p, c, d]: partition = key index within chunk
    vl_ap = vl_d.ap().rearrange("u (c p) d -> u p c d", p=128)
    bias_ap = bias_d.ap()
    biasb_ap = biasb_d.ap()
    ot_ap = ot_d.ap()
    nm_ap = nm_d.ap()

    with tile.TileContext(nc) as tc:
        with (
            tc.tile_pool(name="kt_pool", bufs=2) as kt_pool,
            tc.tile_pool(name="vl_pool", bufs=2) as vl_pool,
            tc.tile_pool(name="vf_pool", bufs=2) as vf_pool,
            tc.tile_pool(name="qt_pool", bufs=2) as qt_pool,
            tc.tile_pool(name="cn_pool", bufs=1) as cn_pool,
            tc.tile_pool(name="e_pool", bufs=4) as e_pool,
            tc.tile_pool(name="ob_pool", bufs=2) as ob_pool,
            tc.tile_pool(name="s_pool", bufs=3, space="PSUM") as s_pool,
            tc.tile_pool(name="o_pool", bufs=1, space="PSUM") as o_pool,
        ):
            bias_t = cn_pool.tile([128, NLC], F32, name="bias_t")
            biasb_t = cn_pool.tile([128, NLC], F32, name="biasb_t")
            ones_t = cn_pool.tile([128, 1], BF16, name="ones_t")
            nc.vector.memset(ones_t[:], 1.0)
            # dependency-free dummy exp: the auto-inserted ACT table load
            # (1.3us) attaches here and runs at t~0 instead of gating the
            # first real exp
            warm_a = cn_pool.tile([128, 1], BF16, name="warm_a")
            nc.scalar.activation(warm_a[:], ones_t[:],
                                 mybir.ActivationFunctionType.Exp,
                                 bias=0.0, scale=1.0)

            def load_bias():
                nc.sync.dma_start(out=bias_t[:], in_=bias_ap)
                if dve_set:
                    # biasb feeds only the DVE exp path
                    nc.sync.dma_start(out=biasb_t[:], in_=biasb_ap)

            import contextlib
            loop_cm = (tc.For_i(0, TIME_LOOP, 1) if TIME_LOOP > 1
                       else contextlib.nullcontext())

            # ablation switches (timing experiments only)
            do_exp = ABLATE not in ("noexp", "empty")
            do_b = ABLATE not in ("nob", "empty")
            do_any = ABLATE != "empty"
            dummy_e = None
            if not do_exp and do_any:
                dummy_e = cn_pool.tile([128, QSPAN], BF16, name="dummy_e")
                nc.vector.memset(dummy_e[:], 0.001)

            loaded = {}

            def load_unit(u, warm=False):
                # qt first (every chunk needs it), then k/v leading pieces
                # (compute starts as soon as they land), then the rest —
                # few DMAs per unit keeps the serialized HWDGE issue cost low
                qt = qt_pool.tile([128, QSPAN], BF16, name=f"qt_u{u}", tag="qt")
                if warm:
                    # halves: the first A matmul only needs qt[:, :512]
                    nc.sync.dma_start(out=qt[:, :512], in_=qt_ap[u][:, :512])
                else:
                    nc.sync.dma_start(out=qt[:], in_=qt_ap[u])
                kt = kt_pool.tile([128, LK], BF16, name=f"kt_u{u}", tag="kt")
                vl = vl_pool.tile([128, NLC, 128], BF16,
                                  name=f"vl_u{u}", tag="vl")
                vf = (vf_pool.tile([128, npair, 2, 128], FP8,
                                   name=f"vf_u{u}", tag="vf")
                      if npair else None)
                if warm:
                    # cold start: the head chain is issue-latency bound, so
                    # the three tiles the first chunk needs (qt half, first
                    # kt piece, bias) issue on three independent HWDGE
                    # queues in parallel.  vl has 3 chunk-iterations of
                    # slack (B lags A by 3).
                    k8, c8 = LK // 8, NLC // 8
                    nc.scalar.dma_start(out=kt[:, :k8], in_=kt_ap[u][:, :k8])
                    nc.scalar.dma_start(out=bias_t[:], in_=bias_ap)
                    if dve_set:
                        nc.scalar.dma_start(out=biasb_t[:], in_=biasb_ap)
                    nc.sync.dma_start(out=qt[:, 512:], in_=qt_ap[u][:, 512:])
                    nc.sync.dma_start(out=vl[:, :c8, :],
                                      in_=vl_ap[u][:, :c8, :])
                    if npair:
                        nc.sync.dma_start(out=vf[:], in_=vf_ap[u])
                    nc.sync.dma_start(out=vl[:, c8:2 * c8, :],
                                      in_=vl_ap[u][:, c8:2 * c8, :])
                    nc.sync.dma_start(out=kt[:, k8:], in_=kt_ap[u][:, k8:])
                    nc.sync.dma_start(out=vl[:, 2 * c8:, :],
                                      in_=vl_ap[u][:, 2 * c8:, :])
                else:
                    k4, c4 = LK // 4, NLC // 4
                    nc.sync.dma_start(out=kt[:, :k4], in_=kt_ap[u][:, :k4])
                    nc.sync.dma_start(out=vl[:, :c4, :],
                                      in_=vl_ap[u][:, :c4, :])
                    if npair:
                        nc.sync.dma_start(out=vf[:], in_=vf_ap[u])
                    nc.sync.dma_start(out=kt[:, k4:], in_=kt_ap[u][:, k4:])
                    nc.sync.dma_start(out=vl[:, c4:, :],
                                      in_=vl_ap[u][:, c4:, :])
                loaded[u] = (kt, vl, qt, vf)

            NG = UNITS_PER_CORE * NLC

            with loop_cm:
                if do_any:
                    load_unit(0, warm=True)
                else:
                    load_bias()
                # One global software-pipelined chunk stream across all
                # units: emit A(g) before B(g-1) so PE's in-order queue
                # always has independent work while the exp(g-1) runs, and
                # the next unit's A-phase fills the previous unit's drain.
                cur, ot_t, nm_t = {}, {}, {}
                etiles, ptiles, qtiles, otiles = {}, {}, {}, {}
                dve_b = {}     # iteration -> (chunk, e tile): deferred B phases
                dve_exp_pend = {}   # iteration -> (chunk, s tile, cg)
                pend_oct = []  # oct reductions deferred past the DVE stretch
                otiles2 = {}   # per-unit final oct (kept out of the acc chain)
                cur_ef = {}    # per-unit open fp8 pair tile
                fp8_b = {}     # iteration -> (pair tile, pair index)
                for g in range(NG + 6 if do_any else 0):
                    if g < NG:
                        ug, cg = g // NLC, g % NLC
                        if cg == 0:
                            cur[ug] = loaded.pop(ug)
                        kt, vl, qt, vf = cur[ug]
                        s = s_pool.tile([128, QSPAN], F32)
                        for half in range(QSPAN // MM_N):
                            sl = bass.ts(half, MM_N)
                            nc.tensor.matmul(
                                s[:, sl], lhsT=kt[:, bass.ts(cg, 128)],
                                rhs=qt[:, sl], start=True, stop=True)
                        is_dve = do_exp and cg in dve_set
                        if is_dve:
                            # exp on DVE (bf16 bits of e^x via mult+add) —
                            # emitted next iteration, after that block's DVE
                            # adds, so the strict-FIFO DVE never idles
                            # waiting for this A matmul to finish.  Its B
                            # matmuls are deferred 3 iterations so the
                            # in-order PE queue never waits on the DVE.
                            dve_exp_pend[g + 1] = (g, s, cg)
                        elif do_exp and cg in fp8_even:
                            # biased pair: exp straight to fp8e4 halves of a
                            # [128, 2, q] pair tile for the DoubleRow B MM
                            ef = e_pool.tile([128, 2, QSPAN], FP8,
                                             tag="ef", name=f"ef_{g}")
                            cur_ef[ug] = ef
                            nc.scalar.activation(
                                ef[:, 0, :], s[:],
                                mybir.ActivationFunctionType.Exp,
                                bias=bias_t[:, cg:cg + 1], scale=SCALE)
                            etiles[g] = ef[:, 0, :]
                        elif do_exp and cg in fp8_odd:
                            ef = cur_ef[ug]
                            nc.scalar.activation(
                                ef[:, 1, :], s[:],
                                mybir.ActivationFunctionType.Exp,
                                bias=bias_t[:, cg:cg + 1], scale=SCALE)
                            etiles[g] = ef[:, 1, :]
                            fp8_b[g + 3] = (ef, pair_idx[cg - 1])
                        elif do_exp:
                            e = e_pool.tile([128, QSPAN], BF16, bufs=6,
                                            tag="e", name=f"e_{g}")
                            nc.scalar.activation(
                                e[:], s[:],
                                mybir.ActivationFunctionType.Exp,
                                bias=bias_t[:, cg:cg + 1], scale=SCALE)
                            etiles[g] = e
                        else:
                            etiles[g] = dummy_e
                        if cg == 8 and ug + 1 < UNITS_PER_CORE:
                            load_unit(ug + 1)  # prefetch next unit's inputs

                    def emit_b(d, e):
                        ud, dl = d // NLC, d % NLC
                        for half in range(QSPAN // MM_N):
                            sl = bass.ts(half, MM_N)
                            nc.tensor.matmul(
                                ot_t[ud][:, sl], lhsT=cur[ud][1][:, dl, :],
                                rhs=e[:, sl],
                                start=(dl == 0), stop=(dl == NLC - 1))

                    # B phases run at lag 3 (the wait-on-exp is long
                    # pre-satisfied when B dispatches); at the end the last
                    # three drain together so the evacuation/norm chain
                    # starts ~2 iterations earlier
                    if g < NG:
                        dues = [g - 3]
                    elif g == NG:
                        dues = [NG - 3, NG - 2, NG - 1]
                    else:
                        dues = []
                    for d in dues:
                     if 0 <= d < NG and do_b:
                        ud, dl = d // NLC, d % NLC
                        if dl == 0:
                            ot_t[ud] = o_pool.tile([128, QSPAN], F32,
                                                   name=f"ot_u{ud}", tag="ot")
                        if not (do_exp and (dl in dve_set or dl in fp8_even
                                            or dl in fp8_odd)):
                            emit_b(d, etiles[d])
                        if g in fp8_b:
                            # one DoubleRow MM pair covers both fp8 chunks:
                            # contraction over (partition, j) = 256 keys at
                            # 2 key-rows/cycle
                            ef, pi = fp8_b.pop(g)
                            for half in range(QSPAN // MM_N):
                                sl = bass.ts(half, MM_N)
                                nc.tensor.matmul(
                                    ot_t[ud][:, sl],
                                    lhsT=cur[ud][3][:, pi, :, :],
                                    rhs=ef[:, :, sl],
                                    start=False, stop=False,
                                    perf_mode=mybir.MatmulPerfMode.DoubleRow)
                        for item in dve_b.pop(g, []):
                            emit_b(*item)
                        if do_exp and dl % 2 == 1:
                            pp = e_pool.tile([128, QSPAN], BF16,
                                             tag="pp", name=f"pp_{d}")
                            nc.vector.tensor_add(
                                pp[:], etiles.pop(d - 1)[:], etiles[d][:])
                            ptiles[d // 2] = pp
                        if do_exp and dl % 4 == 3:
                            qq = e_pool.tile([128, QSPAN], BF16,
                                             tag="qq", bufs=12,
                                             name=f"qq_{d}")
                            nc.vector.tensor_add(
                                qq[:], ptiles.pop(d // 2 - 1)[:],
                                ptiles.pop(d // 2)[:])
                            qtiles[d // 4] = qq

                        if g in dve_exp_pend:
                            gd, sd, cgd = dve_exp_pend.pop(g)
                            e = e_pool.tile([128, QSPAN], BF16,
                                            tag="ed", bufs=12,
                                            name=f"ed_{gd}")
                            nc.vector.tensor_scalar(
                                e[:].bitcast(I16), sd[:],
                                EXP_A * SCALE,
                                biasb_t[:, cgd:cgd + 1],
                                mybir.AluOpType.mult,
                                mybir.AluOpType.add)
                            etiles[gd] = e
                            # B matmuls spread one-per-iteration past the
                            # DVE stretch, keeping in-stretch PE load low
                            kk = sorted(dve_set).index(cgd)
                            dl_tgt = min(max(dve_set) + 1 + 2 * kk, NLC - 2)
                            tgt = max((gd // NLC) * NLC + dl_tgt + 2, gd + 3)
                            dve_b.setdefault(tgt, []).append((gd, e))

                        def emit_oct(dd):
                            udd = dd // NLC
                            oo = e_pool.tile([128, QSPAN], BF16,
                                             tag="oo", name=f"oo_{dd}")
                            nc.vector.tensor_add(
                                oo[:], qtiles.pop(dd // 4 - 1)[:],
                                qtiles.pop(dd // 4)[:])
                            if dd % NLC == 7:
                                otiles[udd] = oo
                            elif dd % NLC == NLC - 1:
                                # last oct stays separate: norm issues one
                                # accumulating matmul pair per operand, so
                                # the first pair runs before the unit ends
                                otiles2[udd] = oo
                            else:
                                # running unit accumulator: acc += oct
                                na = e_pool.tile([128, QSPAN], BF16,
                                                 tag="acc", name=f"acc_{dd}")
                                nc.vector.tensor_add(
                                    na[:], otiles[udd][:], oo[:])
                                otiles[udd] = na

                        if do_exp and dl % 8 == 7:
                            # inside the DVE-exp stretch the DVE is near
                            # saturation: defer oct+acc reduction until after
                            # the stretch (qq tiles ride a deeper ring)
                            if dve_set and (min(dve_set) - 1 <= dl
                                            <= max(dve_set) + 2):
                                pend_oct.append(d)
                            else:
                                emit_oct(d)
                        if (do_exp and pend_oct
                                and (dl > max(dve_set) + 2
                                     or dl >= NLC - 2)):
                            emit_oct(pend_oct.pop(0))
                        if dl == NLC - 1:
                            while do_exp and pend_oct:
                                emit_oct(pend_oct.pop(0))
                            etiles.pop(d)
                            ot = ot_t.pop(ud)
                            ot_sb = ob_pool.tile([128, QSPAN], F32,
                                                 name=f"otsb_u{ud}",
                                                 tag="otsb")
                            # evacuate + store per half: shortens both the
                            # end-of-kernel tail and the WAR stall on the
                            # next unit's first B matmuls
                            for half in range(QSPAN // MM_N):
                                sl = bass.ts(half, MM_N)
                                nc.vector.tensor_scalar_add(
                                    ot_sb[:, sl], ot[:, sl], 0.0)
                                nc.sync.dma_start(out=ot_ap[ud][:, sl],
                                                  in_=ot_sb[:, sl])
                    n1 = g - 3              # unit whose norm part 1 is due
                    if (n1 >= 0 and n1 % NLC == NLC - 2 and n1 < NG
                            and do_b):
                        un = n1 // NLC
                        acc = otiles.pop(un) if do_exp else dummy_e
                        # norm accumulates in an s-ring slot (no dedicated
                        # PSUM bank); part 1 = everything but the last oct,
                        # emitted early so only part 2 sits in the tail
                        nm = s_pool.tile([128, QSPAN], F32, name=f"nm_u{un}",
                                         tag="s")
                        nm_t[un] = nm
                        for half in range(QSPAN // MM_N):
                            sl = bass.ts(half, MM_N)
                            nc.tensor.matmul(
                                nm[0:1, sl], lhsT=ones_t[:],
                                rhs=acc[:, sl], start=True, stop=False)
                    n = g - 4               # unit whose norm part 2 is due
                    if n >= 0 and n % NLC == NLC - 1 and n < NG and do_b:
                        un = n // NLC
                        last = otiles2.pop(un) if do_exp else dummy_e
                        nm = nm_t.pop(un)
                        for half in range(QSPAN // MM_N):
                            sl = bass.ts(half, MM_N)
                            nc.tensor.matmul(
                                nm[0:1, sl], lhsT=ones_t[:],
                                rhs=last[:, sl], start=False, stop=True)
                        nm_sb = ob_pool.tile([1, QSPAN], F32,
                                             name=f"nmsb_u{un}", tag="nmsb")
                        if un == UNITS_PER_CORE - 1:
                            # tail: ACT is idle by now, DVE still drains the
                            # ot evacuation — copy via ACT off the DVE queue
                            nc.scalar.copy(nm_sb[:], nm[0:1, :])
                        else:
                            nc.vector.tensor_scalar_add(
                                nm_sb[:], nm[0:1, :], 0.0)
                        nc.sync.dma_start(out=nm_ap[un], in_=nm_sb[:])

    nc.compile()
    return nc


def _get_program(fs=1536, bs=6144):
    key = (_pick_dve_chunks(fs, bs), _pick_fp8_pairs(fs, bs),
           ABLATE, TIME_LOOP, MM_N)
    if key not in _CACHED:
        _CACHED[key] = _build_program(key[0], key[1])
    return _CACHED[key]


def _host_prep(q, k, v, frame_seqlen, current_block_start):
    fs = max(0, min(int(frame_seqlen), LK))
    bs = max(0, min(int(current_block_start), LK))
    logw = np.zeros(LK, np.float32)
    logw[fs:bs] = math.log(0.1)
    bias = np.ascontiguousarray(logw.reshape(NLC, 128).T)  # [128, NLC]
    biasb = (EXP_B + EXP_A * bias).astype(np.float32)

    q = np.asarray(q, dtype=np.float32)
    k = np.asarray(k, dtype=np.float32)
    v = np.asarray(v, dtype=np.float32)

    qT = np.ascontiguousarray(q[0].transpose(1, 2, 0)).astype(NP_BF16)  # [H,128,LQ]
    kT = np.ascontiguousarray(k[0].transpose(1, 2, 0)).astype(NP_BF16)  # [H,128,LK]
    vL = np.ascontiguousarray(v[0].transpose(1, 0, 2)).astype(NP_BF16)  # [H,LK,128]

    pairs = list(_pick_fp8_pairs(fs, bs))
    vF = None
    if pairs:
        # [H, 128(l), npair, 2, 128(d)] fp8 pair layout for DoubleRow lhsT
        vch = v[0].reshape(NLC, 128, H, D).transpose(2, 0, 1, 3)  # [H,c,l,d]
        vF = np.stack([vch[:, pairs, :, :], vch[:, [c + 1 for c in pairs]]],
                      axis=3)                   # [H, npair, l, 2, d]
        vF = np.ascontiguousarray(vF.transpose(0, 2, 1, 3, 4)).astype(NP_FP8)

    in_maps = []
    for i in range(N_CORES):
        units = [3 * i + uu for uu in range(UNITS_PER_CORE)]
        heads = [g // 2 for g in units]
        qhs = [g % 2 for g in units]
        m = {
            "qt": np.ascontiguousarray(
                np.stack([qT[h, :, qh * QSPAN:(qh + 1) * QSPAN]
                          for h, qh in zip(heads, qhs)])),
            "kt": np.ascontiguousarray(np.stack([kT[h] for h in heads])),
            "vl": np.ascontiguousarray(np.stack([vL[h] for h in heads])),
            "bias": bias,
            "biasb": biasb,
        }
        if pairs:
            m["vf"] = np.ascontiguousarray(np.stack([vF[h] for h in heads]))
        in_maps.append(m)
    return in_maps


def _assemble(results):
    out = np.empty((B, LQ, H, D), np.float32)
    for i in range(N_CORES):
        ot = results[i]["ot"]   # [3, 128, 1024] unnormalized O^T
        nm = results[i]["nm"][:, 0]   # [3, 1024]
        for uu in range(UNITS_PER_CORE):
            g = 3 * i + uu
            h, qh = g // 2, g % 2
            out[0, qh * QSPAN:(qh + 1) * QSPAN, h, :] = (
                ot[uu] / nm[uu][None, :]).T
    return out


def kernel(q, k, v, frame_seqlen, current_block_start):
    fs = max(0, min(int(frame_seqlen), LK))
    bs = max(0, min(int(current_block_start), LK))
    nc = _get_program(fs, bs)
    in_maps = _host_prep(q, k, v, frame_seqlen, current_block_start)
    res = run_bass_kernel_spmd(nc, in_maps, core_ids=list(range(N_CORES)))
    return _assemble(res.results)
